# revision 40
# baseline (speedup 1.0000x reference)
"""Distributed causal multi-head attention for Trainium2 (8 NeuronCores).

Problem: x[2,2048,1024] @ w_qkv[1024,3072] -> 16-head causal attention
         -> @ w_out[1024,1024]. fp32 reference; device compute in bf16
         (fp32 PSUM accumulation), measured end-to-end rel err ~5e-3.

Sharding (8 cores): core c owns heads {2c, 2c+1} for BOTH batches
(feature slice [128c, 128c+128) of each of q/k/v), computes its heads'
attention output attT (feature-major), exchanges via one 8-way AllToAll
so core c ends up with ALL 1024 features for flattened rows
[512c, 512c+512) of (b*2048+i), then computes its [512,1024] slice of
the output projection locally.

Device pipeline per core:
  P1: qT,kT = (w_qk stationary) @ xT chunks   [bf16, N=512 moving]
      vT    = (w_v stationary)  @ xT chunks -> PE-transpose -> V seq-major
      V_aug = [V_h | ones] per head           [ones column => row sums]
      batch 0 runs dt-outer accumulation passes so PE overlaps the xT DMA.
  P2: per (batch, i-chunk of 512): for each causal j-tile:
      S^T[j,i] pair (2 heads row-tiled in PE, K=64 each) -> one ACT exp
      over [128,1024] (scale fused, bf16 out) -> diagonal mask multiply
      -> PV: out^T[65,512] += V_aug.T @ P^T  (row 64 = softmax denom)
      normalize: denom -> SBUF -> partition 0 (SBUF->SBUF DMA) -> recip
      -> gpsimd partition-broadcast -> multiply -> DMA to A2A buffer.
  P3: AllToAll [8,128,512] bf16 -> out[512,1024] = attT_full.T @ w_out
"""
import os
import numpy as np
import ml_dtypes

import concourse.bass as bass
import concourse.bacc as bacc
import concourse.mybir as mybir
import concourse.tile as tile
from concourse.bass_utils import run_bass_kernel_spmd

F32 = mybir.dt.float32
BF16 = mybir.dt.bfloat16
AF = mybir.ActivationFunctionType

NC = 8           # cores
NB = 2           # batches
N = 2048         # seq len
D = 1024         # model dim
HPC = 2          # heads per core
HD = 64          # head dim
FS = HPC * HD    # per-core feature slice (128)
NFLAT = NB * N   # 4096 flattened rows
ROWS = NFLAT // NC   # 512 output rows per core
SCALE = HD ** -0.5

_CACHED_NC = None
DEBUG_TAPS = False


def build_graph():
    nc = bacc.Bacc("TRN2", target_bir_lowering=False, debug=False,
                   num_devices=NC)

    xT = nc.dram_tensor("xT", [NB * 8, 128, N], BF16, kind="ExternalInput")
    wqkv = nc.dram_tensor("wqkv", [8, 128, 3 * FS], BF16, kind="ExternalInput")
    wout = nc.dram_tensor("wout", [8, 128, D], BF16, kind="ExternalInput")
    rankv = nc.dram_tensor("rankv", [1, 1], mybir.dt.int32, kind="ExternalInput")
    mask = nc.dram_tensor("mask", [4, 128, 512], BF16, kind="ExternalInput")
    ident = nc.dram_tensor("ident", [128, 128], BF16, kind="ExternalInput")
    out = nc.dram_tensor("out", [ROWS, D], F32, kind="ExternalOutput")
    dbg = {}
    if DEBUG_TAPS:
        dbg["qkT"] = nc.dram_tensor("dbg_qkT", [128, 2, NFLAT], BF16,
                                    kind="ExternalOutput")
        dbg["vaug"] = nc.dram_tensor("dbg_vaug", [128, 32, HPC, HD + 1], BF16,
                                     kind="ExternalOutput")
        dbg["pt"] = nc.dram_tensor("dbg_pt", [128, 1024], BF16,
                                   kind="ExternalOutput")
        dbg["pv"] = nc.dram_tensor("dbg_pv", [HD + 1, 512], F32,
                                   kind="ExternalOutput")
        dbg["bc"] = nc.dram_tensor("dbg_bc", [HD, 512], F32,
                                   kind="ExternalOutput")
        dbg["an"] = nc.dram_tensor("dbg_an", [HD, 512], BF16,
                                   kind="ExternalOutput")

    with tile.TileContext(nc) as tc:
        _emit(nc, tc, xT, wqkv, wout, mask, ident, rankv, out, dbg)
    nc.compile()
    return nc


def _emit(nc, tc, xT, wqkv, wout, mask, ident, rankv, out, dbg=None):
    dbg = dbg or {}
    ctx_pools = []

    def pool(name, **kw):
        cm = tc.tile_pool(name=name, **kw)
        p = cm.__enter__()
        ctx_pools.append(cm)
        return p

    wpool = pool("weights", bufs=1)
    xpool = pool("xt", bufs=16)
    pinit_cm = tc.tile_pool(name="psum_init", bufs=1, space="PSUM")
    pinit = pinit_cm.__enter__()
    ptpool = pool("pt", bufs=10)
    spool = pool("stage", bufs=1)
    dpool = pool("dram", bufs=1, space="DRAM")

    # ---- persistent SBUF buffers ----
    wqkv_sb = wpool.tile([128, 8, 3 * FS], BF16)
    mask_sb = wpool.tile([128, 4, 512], BF16)
    ident_sb = wpool.tile([128, 128], BF16)
    qkT_sb = wpool.tile([128, 2, NFLAT], BF16)          # [dims, q/k, b*N+i]
    vaug_sb = wpool.tile([128, 32, HPC, HD + 1], BF16)  # per j-tile [V_h|ones]
    attr_sb = wpool.tile([128, 8, 512], BF16)
    wout_sb = wpool.tile([128, 8, D], BF16)

    a2a_in = dpool.tile([NC, FS, ROWS], BF16)
    a2a_out = dpool.tile([NC, FS, ROWS], BF16)

    # weights first (small, contiguous per-tile), then batch-0 xT; batch-1
    # xT and phase-3 weights are emitted after P1 b0 on the ACT queue so
    # they don't steal HBM bandwidth from the critical path.
    xt = {}
    for b in range(NB):
        for dt in range(8):
            xt[b, dt] = xpool.tile([128, N], BF16, tag="xt",
                                   name=f"xt{b}_{dt}")
    for dt in range(8):
        nc.sync.dma_start(wqkv_sb[:, dt, :], wqkv[dt])
        nc.sync.dma_start(xt[0, dt][:], xT[dt])
    rank_sb = wpool.tile([1, 1], mybir.dt.int32)
    nc.sync.dma_start(rank_sb[:], rankv[:])
    for q in range(4):
        nc.sync.dma_start(mask_sb[:, q, :], mask[q])
    nc.sync.dma_start(ident_sb[:], ident[:])
    nc.vector.memset(vaug_sb[:, :, :, HD], 1.0)

    def qk_mm(ps, b, ft, ic, dt):
        nc.tensor.matmul(
            ps[:],
            wqkv_sb[:, dt, 128 * ft:128 * (ft + 1)],
            xt[b, dt][:, 512 * ic:512 * (ic + 1)],
            start=(dt == 0), stop=(dt == 7))

    def vt_mm(ps, b, ic, dt):
        nc.tensor.matmul(
            ps[:],
            wqkv_sb[:, dt, 2 * FS:3 * FS],
            xt[b, dt][:, 512 * ic:512 * (ic + 1)],
            start=(dt == 0), stop=(dt == 7))

    def finish_qk(ps, b, ft, ic):
        nc.vector.tensor_copy(
            qkT_sb[:, ft, b * N + 512 * ic: b * N + 512 * (ic + 1)], ps[:])

    def finish_v(vps_list, b, psum_pool, ptag, pbufs):
        vT_bf = spool.tile([128, N], BF16, tag="vtb", bufs=2, name=f"vtb{b}")
        for ic in range(4):
            nc.vector.tensor_copy(vT_bf[:, 512 * ic:512 * (ic + 1)],
                                  vps_list[ic][:])
        for it in range(16):
            tp = psum_pool.tile([128, 128], BF16, tag=ptag, bufs=pbufs,
                                name=f"t_ps{b}_{it}")
            nc.tensor.transpose(tp[:], vT_bf[:, 128 * it:128 * (it + 1)],
                                ident_sb[:])
            nc.vector.tensor_copy(
                vaug_sb[:, 16 * b + it, :, 0:HD],
                tp[:].rearrange("p (h c) -> p h c", h=HPC))

    # ---- Phase 1, batch 0: dt-outer passes (overlap the xT DMA) ----
    b = 0
    qk_ps = {(ft, ic): pinit.tile([128, 512], F32, tag="init",
                                  bufs=8, name=f"qk0_{ft}_{ic}")
             for ft in range(2) for ic in range(4)}
    for dt in range(8):
        for ft in range(2):
            for ic in range(4):
                qk_mm(qk_ps[ft, ic], 0, ft, ic, dt)
    for ft in range(2):
        for ic in range(4):
            finish_qk(qk_ps[ft, ic], 0, ft, ic)
    v_ps0 = [pinit.tile([128, 512], F32, tag="init", bufs=8,
                        name=f"v0_{ic}") for ic in range(4)]
    for dt in range(8):
        for ic in range(4):
            vt_mm(v_ps0[ic], 0, ic, dt)
    finish_v(v_ps0, 0, pinit, "init", 8)
    pinit_cm.__exit__(None, None, None)
    ppool = pool("psum", bufs=1, space="PSUM")
    for dt in range(8):
        nc.sync.dma_start(xt[1, dt][:], xT[8 + dt])
    for dt in range(8):
        nc.sync.dma_start(wout_sb[:, dt, :], wout[dt])

    def phase1_seq(b):
        # dt-inner (xT already resident); 2 psum slots suffice
        for ft in range(2):
            for ic in range(4):
                ps = ppool.tile([128, 512], F32, tag="mm", bufs=2,
                                name=f"qk_ps{b}_{ft}_{ic}")
                for dt in range(8):
                    qk_mm(ps, b, ft, ic, dt)
                finish_qk(ps, b, ft, ic)
        vps = []
        for ic in range(4):
            ps = ppool.tile([128, 512], F32, tag="mm", bufs=2,
                            name=f"v_ps{b}_{ic}")
            for dt in range(8):
                vt_mm(ps, b, ic, dt)
            vps.append(ps)
        finish_v(vps, b, ppool, "mm", 2)

    def phase2(b):
        for ic in range(4):
            pv = [ppool.tile([HD + 1, 512], F32, tag="pv", bufs=2,
                             name=f"pv{b}_{ic}_{h}") for h in range(HPC)]
            njt = 4 * ic + 4
            for jt in range(njt):
                jglob = 16 * b + jt
                # diagonal tile q: columns < 128q are entirely masked out
                q = jt - 4 * ic
                c0 = 128 * q if q > 0 else 0
                W = 512 - c0
                s_ps = ppool.tile([128, 1024], F32, tag="s", bufs=2,
                                  name=f"s{b}_{ic}_{jt}")
                pt = ptpool.tile([128, 1024], BF16, tag="pt",
                                 name=f"pt{b}_{ic}_{jt}")
                for h in range(HPC):
                    nc.tensor.matmul(
                        s_ps[:, 512 * h + c0:512 * (h + 1)],
                        qkT_sb[64 * h:64 * (h + 1), 1,
                               b * N + 128 * jt: b * N + 128 * (jt + 1)],
                        qkT_sb[64 * h:64 * (h + 1), 0,
                               b * N + 512 * ic + c0: b * N + 512 * (ic + 1)],
                        start=True, stop=True)
                s3 = s_ps[:].rearrange("p (h f) -> p h f", h=HPC)
                pt3 = pt[:].rearrange("p (h f) -> p h f", h=HPC)
                nc.scalar.activation(pt3[:, :, c0:512], s3[:, :, c0:512],
                                     AF.Exp, scale=SCALE)
                if q >= 0:
                    nc.vector.tensor_mul(
                        pt3[:, :, c0:512],
                        pt3[:, :, c0:512],
                        mask_sb[:, q:q + 1, c0:512].to_broadcast(
                            (128, HPC, W)))
                if b == 0 and ic == 0 and jt == 0 and "pt" in dbg:
                    nc.sync.dma_start(dbg["pt"][:], pt[:])
                for h in range(HPC):
                    nc.tensor.matmul(
                        pv[h][:, c0:512],
                        vaug_sb[:, jglob, h, :],
                        pt[:, 512 * h + c0:512 * (h + 1)],
                        start=(jt == 0), stop=(jt == njt - 1))
            for h in range(HPC):
                # move denom row to partition 0: aligned DVE copy out of
                # PSUM, then SBUF->SBUF DMA (DVE ops can't shift partitions)
                sum64 = spool.tile([HD + 1, 512], F32, tag="sum64", bufs=2,
                                   name=f"s64_{b}_{ic}_{h}")
                nc.vector.tensor_copy(sum64[HD:HD + 1, :], pv[h][HD:HD + 1, :])
                sums = spool.tile([1, 512], F32, tag="sums", bufs=2,
                                  name=f"sm{b}_{ic}_{h}")
                nc.sync.dma_start(sums[:], sum64[HD:HD + 1, :])
                recip = spool.tile([1, 512], F32, tag="recip", bufs=2,
                                   name=f"rc{b}_{ic}_{h}")
                nc.vector.reciprocal_approx_fast(recip[:], sums[:])
                bc = spool.tile([HD, 512], F32, tag="bc", bufs=2,
                                name=f"bc{b}_{ic}_{h}")
                nc.gpsimd.partition_broadcast(bc[:], recip[:])
                an = spool.tile([HD, 512], BF16, tag="an", bufs=4,
                                name=f"an{b}_{ic}_{h}")
                nc.vector.tensor_mul(an[:], pv[h][0:HD, :], bc[:])
                if b == 0 and ic == 0 and h == 0 and "pv" in dbg:
                    pvc = spool.tile([HD + 1, 512], F32, tag="pvc", name="pvc")
                    nc.vector.tensor_copy(pvc[:], pv[h][:])
                    nc.sync.dma_start(dbg["pv"][:], pvc[:])
                    nc.sync.dma_start(dbg["bc"][:], bc[:])
                    nc.sync.dma_start(dbg["an"][:], an[:])
                nc.sync.dma_start(
                    a2a_in[4 * b + ic, HD * h:HD * (h + 1), :], an[:])

    phase2(0)
    if "qkT" in dbg:
        nc.sync.dma_start(dbg["qkT"][:], qkT_sb[:])
        nc.sync.dma_start(dbg["vaug"][:], vaug_sb[:])
    phase1_seq(1)
    phase2(1)

    # ================= Phase 3 =================
    nc.gpsimd.collective_compute(
        "AllToAll", mybir.AluOpType.bypass,
        replica_groups=[list(range(NC))],
        ins=[a2a_in.opt()], outs=[a2a_out.opt()])
    nc.sync.dma_start(attr_sb[:],
                      a2a_out[:].rearrange("s p i -> p s i"))
    for it in range(4):
        for oc in range(2):
            ps = ppool.tile([128, 512], F32, tag="mm", bufs=2,
                            name=f"op_ps{it}_{oc}")
            for kt in range(8):
                nc.tensor.matmul(
                    ps[:],
                    attr_sb[:, kt, 128 * it:128 * (it + 1)],
                    wout_sb[:, kt, 512 * oc:512 * (oc + 1)],
                    start=(kt == 0), stop=(kt == 7))
            ob = spool.tile([128, 512], F32, tag="ob", bufs=2,
                            name=f"ob{it}_{oc}")
            nc.vector.tensor_copy(ob[:], ps[:])
            nc.sync.dma_start(
                out[128 * it:128 * (it + 1), 512 * oc:512 * (oc + 1)], ob[:])

    for p in reversed(ctx_pools):
        p.__exit__(None, None, None)


def _host_inputs(x, w_qkv, w_out):
    x = np.asarray(x, dtype=np.float32)
    w_qkv = np.asarray(w_qkv, dtype=np.float32)
    w_out = np.asarray(w_out, dtype=np.float32)

    xT = np.ascontiguousarray(x.reshape(NFLAT, D).T).astype(ml_dtypes.bfloat16)
    # pre-tiled [b*8+dt, p, i] so every load is one contiguous DMA
    xTt = np.ascontiguousarray(
        xT.reshape(8, 128, NB, N).transpose(2, 0, 1, 3).reshape(NB * 8, 128, N))
    wq, wk, wv = w_qkv[:, 0:D], w_qkv[:, D:2 * D], w_qkv[:, 2 * D:3 * D]
    w_out_bf = np.ascontiguousarray(
        w_out.astype(ml_dtypes.bfloat16).reshape(8, 128, D))

    # causal masks for the 4 diagonal j-tiles of each 512-wide i-chunk:
    # keep iff f >= p + 128*q
    p = np.arange(128)[:, None]
    f = np.arange(512)[None, :]
    masks = np.stack([(f >= p + 128 * q) for q in range(4)])
    masks = masks.astype(ml_dtypes.bfloat16)
    identity = np.eye(128, dtype=ml_dtypes.bfloat16)

    in_maps = []
    for c in range(NC):
        sl = slice(FS * c, FS * (c + 1))
        wq_c = np.concatenate([wq[:, sl], wk[:, sl], wv[:, sl]], axis=1)
        wq_c = np.ascontiguousarray(
            wq_c.astype(ml_dtypes.bfloat16).reshape(8, 128, 3 * FS))

        in_maps.append({
            "xT": xTt,
            "wqkv": wq_c,
            "wout": w_out_bf,
            "mask": masks,
            "ident": identity,
            "rankv": np.array([[c]], np.int32),
        })
    return in_maps


def run_hw(inputs, trace=False, **kw):
    """Run on 8 NeuronCores. Returns (full_output, BassKernelResults)."""
    global _CACHED_NC
    if _CACHED_NC is None:
        _CACHED_NC = build_graph()
    in_maps = _host_inputs(inputs["x"], inputs["w_qkv"], inputs["w_out"])
    res = run_bass_kernel_spmd(_CACHED_NC, in_maps,
                               core_ids=list(range(NC)), trace=trace, **kw)
    y = np.concatenate([np.asarray(res.results[c]["out"]) for c in range(NC)],
                       axis=0).reshape(NB, N, D).astype(np.float32)
    return y, res


def kernel(**inputs):
    y, _ = run_hw(inputs, trace=bool(os.environ.get("BASS_TRACE")))
    return y
